# revision 1
# baseline (speedup 1.0000x reference)
"""Bahdanau-attention kernel for Trainium2 (8 NeuronCores, data-parallel over batch).

Computation (per batch b):
    enc_proj = h_enc @ W1.T + b1          # (L, D)   -- the big matmul
    dec_proj = h_dec @ W2.T + b2          # (D,)
    h        = tanh(enc_proj + dec_proj)  # (L, D)
    scores   = h @ V (+ bv)               # (L,)  -- bv cancels in softmax, dropped
    attn     = softmax(scores)            # no-max softmax: |scores| <= ||V||_1 ~ 16, exp is safe
    ctx      = attn @ enc_proj            # (D,)

Device layout: everything transposed ("T-space", e/d on partitions):
  - enc_projT[e, l] accumulated in PSUM via lhsT=W1T tiles, rhs=h_encT tiles
  - h_encT produced by one xbar DMA-transpose per half-batch (fp16)
  - tanh fused with (b1+dec_proj) bias on ACT; exp fused with Z-sum on ACT
  - scores via PE with V replicated to 128 rows -> replicated scores for free
  - ctx via DVE tensor_tensor_reduce against evacuated enc_projT (fp16)
  - divide by Z only at the very end (softmax normalizer cancels until then)
"""

import numpy as np

B, L, D = 32, 2048, 1024
NCORES = 8
NB = B // NCORES  # batches per core
P = 128
NCH = D // P      # 8 chunks of the d/e dimension
NH = 2            # l-halves per batch
LH = L // NH      # 1024

_cache = {}


def _build(reps=1):
    import concourse.bass as bass
    import concourse.tile as tile
    from concourse import bacc, mybir
    from concourse.bass import ts, ds
    from contextlib import ExitStack

    FP16 = mybir.dt.float16
    FP32 = mybir.dt.float32
    Alu = mybir.AluOpType
    Act = mybir.ActivationFunctionType
    X = mybir.AxisListType.X

    nc = bacc.Bacc("TRN2", name="bahdanau_attn")

    h_enc = nc.dram_tensor("h_enc", [NB, L, D], FP32, kind="ExternalInput")
    w1t = nc.dram_tensor("w1t", [NCH, P, D], FP16, kind="ExternalInput")      # [dchunk, dpart, e]
    w2t = nc.dram_tensor("w2t", [NCH, P, D], FP16, kind="ExternalInput")
    hdect = nc.dram_tensor("hdect", [NCH, P, NB], FP16, kind="ExternalInput")  # [dchunk, dpart, b]
    b1t = nc.dram_tensor("b1t", [P, NCH], FP32, kind="ExternalInput")          # b1 as [p, chunk]
    b12t = nc.dram_tensor("b12t", [P, NCH], FP32, kind="ExternalInput")        # (b1+b2) as [p, chunk]
    vt = nc.dram_tensor("vt", [P, NCH], FP16, kind="ExternalInput")            # V as [p, chunk]
    out = nc.dram_tensor("ctx_out", [NB, P, NCH], FP32, kind="ExternalOutput")

    with tile.TileContext(nc) as tc, ExitStack() as ctx:
        wp = ctx.enter_context(tc.tile_pool(name="weights", bufs=1))
        ld = ctx.enter_context(tc.tile_pool(name="loads", bufs=2))
        tp = ctx.enter_context(tc.tile_pool(name="hT", bufs=3))
        ep = ctx.enter_context(tc.tile_pool(name="encproj", bufs=2))
        hp = ctx.enter_context(tc.tile_pool(name="htan", bufs=3))
        xp = ctx.enter_context(tc.tile_pool(name="exps", bufs=2))
        sp = ctx.enter_context(tc.tile_pool(name="scratch", bufs=2))
        fin = ctx.enter_context(tc.tile_pool(name="final", bufs=2))
        psA = ctx.enter_context(tc.tile_pool(name="psA", bufs=2, space="PSUM"))
        psS = ctx.enter_context(tc.tile_pool(name="psS", bufs=2, space="PSUM"))

        # ---- prologue: weights + dec_proj bias ----
        # Per-chunk weight tiles so the first matmuls/dec-proj aren't gated on
        # one monolithic 2MB DMA. w2 first (dec_proj needs it immediately).
        w2_sb = [wp.tile([P, D], FP16, tag=f"w2_{d}", name=f"w2_{d}") for d in range(NCH)]
        for d in range(NCH):
            nc.scalar.dma_start(w2_sb[d], w2t[d])
        hdec_sb = wp.tile([P, NCH, NB], FP16)
        nc.scalar.dma_start(hdec_sb, hdect[:].rearrange("c p b -> p c b"))
        b1_sb = wp.tile([P, NCH], FP32)
        nc.scalar.dma_start(b1_sb, b1t[:])
        b12_sb = wp.tile([P, NCH], FP32)
        nc.scalar.dma_start(b12_sb, b12t[:])
        v_sb = wp.tile([P, NCH], FP16)
        nc.scalar.dma_start(v_sb, vt[:])
        w1_sb = [wp.tile([P, D], FP16, tag=f"w1_{d}", name=f"w1_{d}") for d in range(NCH)]
        for d in range(NCH):
            nc.scalar.dma_start(w1_sb[d], w1t[d])

        # V replicated along a 128-wide M dim so the scores matmul outputs
        # 128 identical rows (replicated scores; Z then comes out per-partition).
        vrep = wp.tile([P, NCH, P], FP16)
        nc.vector.tensor_copy(vrep, v_sb[:, :, None].to_broadcast([P, NCH, P]))

        # bias_sb[:, c, b] = dec_proj[b, e] + b1[e] + b2[e]   (e = c*128 + p)
        bias_sb = wp.tile([P, NCH, NB], FP32)
        for c in range(NCH):
            psd = psA.tile([P, LH], FP32, tag="mm")
            for d in range(NCH):
                nc.tensor.matmul(
                    psd[:, :NB],
                    lhsT=w2_sb[d][:, ts(c, P)],
                    rhs=hdec_sb[:, d, :],
                    start=(d == 0),
                    stop=(d == NCH - 1),
                )
            nc.vector.tensor_scalar(
                out=bias_sb[:, c, :], in0=psd[:, :NB],
                scalar1=b12_sb[:, c : c + 1], scalar2=None, op0=Alu.add,
            )

        # ---- main loop over batches ----
        for _rep in range(reps):
          for b in range(NB):
              exp_rep = xp.tile([P, L], FP16, tag="exp")     # exp(scores), replicated on all partitions
              zsl = fin.tile([P, NH], FP32, tag="zsl")       # per-half sum of exp(scores)
              ctx_sl = fin.tile([P, NCH, NH], FP16, tag="ctxsl")  # per-half ctx partials
              enc_sb = ep.tile([P, NCH, L], FP16, tag="enc")  # enc_projT (with b1), fp16

              for h in range(NH):
                  # load fp32 -> fp16 (SWDGE cast-DMA), [lpart, lchunk, d]
                  nat = ld.tile([P, NH * 4, D], FP16, tag="nat")
                  nc.gpsimd.dma_start(
                      nat, h_enc[b, ds(h * LH, LH), :].rearrange("(t p) d -> p t d", p=P)
                  )
                  # one xbar transpose per half: [l, (t d)] -> [dpart, (t dchunk), l]
                  hT = tp.tile([P, NH * 4, NCH, P], FP16, tag="hT")  # [dpart, t, dchunk, l128]
                  nc.sync.dma_start(hT, nat.rearrange("p t d -> p (t d)"), transpose=True)

                  ps_sc = psS.tile([P, LH], FP32, tag="sc")
                  for c in range(NCH):
                      ps = psA.tile([P, LH], FP32, tag="mm")
                      for g in range(2):  # matmul out must stay within one PSUM bank
                          for d in range(NCH):
                              nc.tensor.matmul(
                                  ps[:, ts(g, LH // 2)],
                                  lhsT=w1_sb[d][:, ts(c, P)],
                                  rhs=hT[:, ds(g * 4, 4), d, :],
                                  start=(d == 0),
                                  stop=(d == NCH - 1),
                              )
                      # tanh(enc_projT + dec_proj + b1 + b2) on ACT, fused bias
                      htan = hp.tile([P, LH], FP16, tag="htan")
                      nc.scalar.activation(htan, ps, Act.Tanh, bias=bias_sb[:, c, b : b + 1])
                      # evacuate enc_projT + b1 to fp16 SBUF (ACT only: keeps the
                      # PSUM-drain path off DVE, whose ctx bursts would stall PE)
                      dst = enc_sb[:, c, ds(h * LH, LH)]
                      nc.scalar.activation(dst, ps, Act.Identity, bias=b1_sb[:, c : c + 1])
                      # scores accumulation on PE (output replicated over 128 rows)
                      for g in range(2):
                          nc.tensor.matmul(
                              ps_sc[:, ts(g, LH // 2)], lhsT=vrep[:, c, :],
                              rhs=htan[:, ts(g, LH // 2)],
                              start=(c == 0), stop=(c == NCH - 1),
                          )
                  # exp(scores) + per-partition Z sum, fused on ACT
                  nc.scalar.activation(
                      exp_rep[:, ds(h * LH, LH)], ps_sc, Act.Exp,
                      accum_out=zsl[:, h : h + 1],
                  )
                  # ctx_unnorm[c] partial for this half (overlaps next half's MMs)
                  with nc.allow_low_precision("fp16 half-partials; |ctx_unnorm|<~1e3"):
                      for c in range(NCH):
                          scratch = sp.tile([P, LH], FP16, tag="ttr")
                          nc.vector.tensor_tensor(
                              scratch, enc_sb[:, c, ds(h * LH, LH)],
                              exp_rep[:, ds(h * LH, LH)], Alu.mult,
                          )
                          nc.vector.tensor_reduce(
                              ctx_sl[:, c, h : h + 1], scratch, axis=X, op=Alu.add
                          )

              # finalize: ctx = ctx_unnorm / Z
              zsum = fin.tile([P, 1], FP32, tag="zsum")
              nc.vector.tensor_reduce(zsum, zsl, axis=X, op=Alu.add)
              recip = fin.tile([P, 1], FP32, tag="recip")
              nc.vector.reciprocal(recip, zsum)
              ctxf = fin.tile([P, NCH], FP32, tag="ctxf")
              ctxr = fin.tile([P, NCH], FP32, tag="ctxr")
              nc.vector.tensor_reduce(ctxr, ctx_sl, axis=X, op=Alu.add)
              nc.vector.tensor_scalar(
                  out=ctxf, in0=ctxr, scalar1=recip, scalar2=None, op0=Alu.mult
              )
              nc.scalar.dma_start(out[b], ctxf)

    nc.finalize()
    return nc


def _prep_shared(W1, b1, W2, b2, V):
    f16 = np.float16
    w1t = np.ascontiguousarray(W1.T.reshape(NCH, P, D).astype(f16))
    w2t = np.ascontiguousarray(W2.T.reshape(NCH, P, D).astype(f16))
    b1t = np.ascontiguousarray(b1.reshape(NCH, P).T.astype(np.float32))
    b12t = np.ascontiguousarray((b1 + b2).reshape(NCH, P).T.astype(np.float32))
    vt = np.ascontiguousarray(V.reshape(NCH, P).T.astype(f16))
    return w1t, w2t, b1t, b12t, vt


def kernel(h_enc, h_dec, W1, b1, W2, b2, V, bv):
    from concourse.bass_utils import run_bass_kernel_spmd

    h_enc = np.asarray(h_enc, dtype=np.float32)
    h_dec = np.asarray(h_dec, dtype=np.float32)
    W1 = np.asarray(W1, dtype=np.float32)
    b1 = np.asarray(b1, dtype=np.float32)
    W2 = np.asarray(W2, dtype=np.float32)
    b2 = np.asarray(b2, dtype=np.float32)
    V = np.asarray(V, dtype=np.float32)

    if "nc" not in _cache:
        _cache["nc"] = _build()
    nc = _cache["nc"]

    w1t, w2t, b1t, b12t, vt = _prep_shared(W1, b1, W2, b2, V)

    in_maps = []
    for core in range(NCORES):
        sl = slice(core * NB, (core + 1) * NB)
        hdect = np.ascontiguousarray(h_dec[sl].T.reshape(NCH, P, NB).astype(np.float16))
        in_maps.append(
            {
                "h_enc": np.ascontiguousarray(h_enc[sl]),
                "w1t": w1t,
                "w2t": w2t,
                "hdect": hdect,
                "b1t": b1t,
                "b12t": b12t,
                "vt": vt,
            }
        )

    res = run_bass_kernel_spmd(nc, in_maps, core_ids=list(range(NCORES)))
    outs = []
    for core in range(NCORES):
        o = res.results[core]["ctx_out"]  # [NB, P, NCH]
        outs.append(o.transpose(0, 2, 1).reshape(NB, D))  # e = c*128 + p
    return np.concatenate(outs, axis=0).astype(np.float32)



# revision 4
# speedup vs baseline: 1.0534x; 1.0534x over previous
"""Bahdanau-attention kernel for Trainium2 (8 NeuronCores, data-parallel over batch).

Computation (per batch b):
    enc_proj = h_enc @ W1.T + b1          # (L, D)   -- the big matmul
    dec_proj = h_dec @ W2.T + b2          # (D,)
    h        = tanh(enc_proj + dec_proj)  # (L, D)
    scores   = h @ V (+ bv)               # (L,)  -- bv cancels in softmax, dropped
    attn     = softmax(scores)            # no-max softmax: |scores| <= ||V||_1 ~ 16, exp is safe
    ctx      = attn @ enc_proj            # (D,)

Device layout: everything transposed ("T-space", e/d on partitions):
  - h_enc cast to fp16 on HOST; device loads are plain (non-cast) DMAs
  - L processed in quarters (512) for fine-grained pipelining:
      nat load (gpsimd ring) -> 2 xbar transpose pieces (sync ring) ->
      8x8 matmuls (PE) -> tanh+evac (ACT) -> scores (PE) -> exp (ACT) ->
      ctx partial (DVE, fused tensor_tensor_reduce)
  - enc_projT accumulated in PSUM via lhsT=W1T tiles, rhs=h_encT tiles
  - tanh fused with (b1+dec_proj) bias on ACT; exp fused with Z-sum on ACT
  - scores via PE with V replicated to 128 rows -> replicated scores for free
  - ctx via one fused DVE tensor_tensor_reduce per (chunk, quarter)
  - divide by Z only at the very end (softmax normalizer cancels until then)
"""

import numpy as np

B, L, D = 32, 2048, 1024
NCORES = 8
NB = B // NCORES  # batches per core
P = 128
NCH = D // P      # 8 chunks of the d/e dimension
NQ = 4            # l-quarters per batch
LQ = L // NQ      # 512
TQ = LQ // P      # 4 l-subtiles per quarter

_cache = {}


def _build(reps=1):
    import concourse.bass as bass
    import concourse.tile as tile
    from concourse import bacc, mybir
    from concourse.bass import ts, ds
    from contextlib import ExitStack

    FP16 = mybir.dt.float16
    FP32 = mybir.dt.float32
    Alu = mybir.AluOpType
    Act = mybir.ActivationFunctionType
    X = mybir.AxisListType.X

    nc = bacc.Bacc("TRN2", name="bahdanau_attn")

    h16 = nc.dram_tensor("h16", [NB, L, D], FP16, kind="ExternalInput")
    w1t = nc.dram_tensor("w1t", [NCH, P, D], FP16, kind="ExternalInput")      # [dchunk, dpart, e]
    w2t = nc.dram_tensor("w2t", [NCH, P, D], FP16, kind="ExternalInput")
    hdect = nc.dram_tensor("hdect", [NCH, P, NB], FP16, kind="ExternalInput")  # [dchunk, dpart, b]
    b1t = nc.dram_tensor("b1t", [P, NCH], FP32, kind="ExternalInput")          # b1 as [p, chunk]
    b12t = nc.dram_tensor("b12t", [P, NCH], FP32, kind="ExternalInput")        # (b1+b2) as [p, chunk]
    vt = nc.dram_tensor("vt", [P, NCH], FP16, kind="ExternalInput")            # V as [p, chunk]
    out = nc.dram_tensor("ctx_out", [NB, P, NCH], FP32, kind="ExternalOutput")

    with tile.TileContext(nc) as tc, ExitStack() as ctx:
        wp = ctx.enter_context(tc.tile_pool(name="weights", bufs=1))
        ld = ctx.enter_context(tc.tile_pool(name="loads", bufs=3))
        tp = ctx.enter_context(tc.tile_pool(name="hT", bufs=4))
        ep = ctx.enter_context(tc.tile_pool(name="encproj", bufs=2))
        hp = ctx.enter_context(tc.tile_pool(name="htan", bufs=4))
        xp = ctx.enter_context(tc.tile_pool(name="exps", bufs=2))
        sp = ctx.enter_context(tc.tile_pool(name="scratch", bufs=3))
        fin = ctx.enter_context(tc.tile_pool(name="final", bufs=2))
        psA = ctx.enter_context(tc.tile_pool(name="psA", bufs=4, space="PSUM"))
        psS = ctx.enter_context(tc.tile_pool(name="psS", bufs=2, space="PSUM"))

        # ---- prologue: weights + dec_proj bias ----
        # w1 first: the first main matmul needs all 8 d-chunk tiles of it.
        w1_sb = [wp.tile([P, D], FP16, tag=f"w1_{d}", name=f"w1_{d}") for d in range(NCH)]
        for d in range(NCH):
            nc.scalar.dma_start(w1_sb[d], w1t[d])
        w2_sb = [wp.tile([P, D], FP16, tag=f"w2_{d}", name=f"w2_{d}") for d in range(NCH)]
        for d in range(NCH):
            nc.scalar.dma_start(w2_sb[d], w2t[d])
        hdec_sb = wp.tile([P, NCH, NB], FP16)
        nc.scalar.dma_start(hdec_sb, hdect[:].rearrange("c p b -> p c b"))
        b1_sb = wp.tile([P, NCH], FP32)
        nc.scalar.dma_start(b1_sb, b1t[:])
        b12_sb = wp.tile([P, NCH], FP32)
        nc.scalar.dma_start(b12_sb, b12t[:])
        v_sb = wp.tile([P, NCH], FP16)
        nc.scalar.dma_start(v_sb, vt[:])

        # V replicated along a 128-wide M dim so the scores matmul outputs
        # 128 identical rows (replicated scores; Z then comes out per-partition).
        vrep = wp.tile([P, NCH, P], FP16)
        nc.vector.tensor_copy(vrep, v_sb[:, :, None].to_broadcast([P, NCH, P]))

        # bias_sb[:, c, b] = dec_proj[b, e] + b1[e] + b2[e]   (e = c*128 + p)
        bias_sb = wp.tile([P, NCH, NB], FP32)
        for c in range(NCH):
            psd = psA.tile([P, LQ], FP32, tag="mm")
            for d in range(NCH):
                nc.tensor.matmul(
                    psd[:, :NB],
                    lhsT=w2_sb[d][:, ts(c, P)],
                    rhs=hdec_sb[:, d, :],
                    start=(d == 0),
                    stop=(d == NCH - 1),
                )
            nc.vector.tensor_scalar(
                out=bias_sb[:, c, :], in0=psd[:, :NB],
                scalar1=b12_sb[:, c : c + 1], scalar2=None, op0=Alu.add,
            )

        # ---- main loop over batches ----
        for _rep in range(reps):
          for b in range(NB):
              exp_rep = xp.tile([P, L], FP16, tag="exp")     # exp(scores), replicated on all partitions
              zsl = fin.tile([P, NQ], FP32, tag="zsl")       # per-quarter sum of exp(scores)
              ctx_sl = fin.tile([P, NCH, NQ], FP32, tag="ctxsl")  # per-quarter ctx partials
              enc_sb = ep.tile([P, NCH, L], FP16, tag="enc")  # enc_projT (with b1), fp16

              for q in range(NQ):
                  # plain fp16 load, [lpart, lsubtile, d] (gpsimd ring)
                  nat = ld.tile([P, TQ, D], FP16, tag="nat")
                  nc.gpsimd.dma_start(
                      nat, h16[b, ds(q * LQ, LQ), :].rearrange("(t p) d -> p t d", p=P)
                  )
                  # one xbar transpose per quarter: [l, (t d)] -> [dpart, (t dchunk), l]
                  hT = tp.tile([P, TQ, NCH, P], FP16, tag="hT")  # [dpart, t, dchunk, l128]
                  nc.sync.dma_start(
                      hT, nat.rearrange("p t d -> p (t d)"), transpose=True
                  )

                  ps_sc = psS.tile([P, LQ], FP32, tag="sc")
                  for c in range(NCH):
                      ps = psA.tile([P, LQ], FP32, tag="mm")
                      for d in range(NCH):
                          nc.tensor.matmul(
                              ps,
                              lhsT=w1_sb[d][:, ts(c, P)],
                              rhs=hT[:, :, d, :],
                              start=(d == 0),
                              stop=(d == NCH - 1),
                          )
                      # tanh(enc_projT + dec_proj + b1 + b2) on ACT, fused bias
                      htan = hp.tile([P, LQ], FP16, tag="htan")
                      nc.scalar.activation(htan, ps, Act.Tanh, bias=bias_sb[:, c, b : b + 1])
                      # evacuate enc_projT + b1 to fp16 SBUF (ACT only: keeps the
                      # PSUM-drain path off DVE, whose ctx bursts would stall PE)
                      dst = enc_sb[:, c, ds(q * LQ, LQ)]
                      nc.scalar.activation(dst, ps, Act.Identity, bias=b1_sb[:, c : c + 1])
                      # scores accumulation on PE (output replicated over 128 rows)
                      nc.tensor.matmul(
                          ps_sc, lhsT=vrep[:, c, :], rhs=htan,
                          start=(c == 0), stop=(c == NCH - 1),
                      )
                  # exp(scores) + per-partition Z sum, fused on ACT
                  nc.scalar.activation(
                      exp_rep[:, ds(q * LQ, LQ)], ps_sc, Act.Exp,
                      accum_out=zsl[:, q : q + 1],
                  )
                  # ctx_unnorm[c] partial for this quarter (overlaps next quarter's MMs)
                  with nc.allow_low_precision("fp16 product scratch; |ctx_unnorm|<~1e3"):
                      for c in range(NCH):
                          scratch = sp.tile([P, LQ], FP16, tag="ttr")
                          nc.vector.tensor_tensor(
                              scratch, enc_sb[:, c, ds(q * LQ, LQ)],
                              exp_rep[:, ds(q * LQ, LQ)], Alu.mult,
                          )
                          nc.vector.tensor_reduce(
                              ctx_sl[:, c, q : q + 1], scratch, axis=X, op=Alu.add
                          )

              # finalize: ctx = ctx_unnorm / Z
              zsum = fin.tile([P, 1], FP32, tag="zsum")
              nc.vector.tensor_reduce(zsum, zsl, axis=X, op=Alu.add)
              recip = fin.tile([P, 1], FP32, tag="recip")
              nc.vector.reciprocal(recip, zsum)
              ctxf = fin.tile([P, NCH], FP32, tag="ctxf")
              ctxr = fin.tile([P, NCH], FP32, tag="ctxr")
              nc.vector.tensor_reduce(ctxr, ctx_sl, axis=X, op=Alu.add)
              nc.vector.tensor_scalar(
                  out=ctxf, in0=ctxr, scalar1=recip, scalar2=None, op0=Alu.mult
              )
              nc.scalar.dma_start(out[b], ctxf)

    nc.finalize()
    return nc


def _prep_shared(W1, b1, W2, b2, V):
    f16 = np.float16
    w1t = np.ascontiguousarray(W1.T.reshape(NCH, P, D).astype(f16))
    w2t = np.ascontiguousarray(W2.T.reshape(NCH, P, D).astype(f16))
    b1t = np.ascontiguousarray(b1.reshape(NCH, P).T.astype(np.float32))
    b12t = np.ascontiguousarray((b1 + b2).reshape(NCH, P).T.astype(np.float32))
    vt = np.ascontiguousarray(V.reshape(NCH, P).T.astype(f16))
    return w1t, w2t, b1t, b12t, vt


def kernel(h_enc, h_dec, W1, b1, W2, b2, V, bv):
    from concourse.bass_utils import run_bass_kernel_spmd

    h_enc = np.asarray(h_enc, dtype=np.float32)
    h_dec = np.asarray(h_dec, dtype=np.float32)
    W1 = np.asarray(W1, dtype=np.float32)
    b1 = np.asarray(b1, dtype=np.float32)
    W2 = np.asarray(W2, dtype=np.float32)
    b2 = np.asarray(b2, dtype=np.float32)
    V = np.asarray(V, dtype=np.float32)

    if "nc" not in _cache:
        _cache["nc"] = _build()
    nc = _cache["nc"]

    w1t, w2t, b1t, b12t, vt = _prep_shared(W1, b1, W2, b2, V)
    h16_full = h_enc.astype(np.float16)

    in_maps = []
    for core in range(NCORES):
        sl = slice(core * NB, (core + 1) * NB)
        hdect = np.ascontiguousarray(h_dec[sl].T.reshape(NCH, P, NB).astype(np.float16))
        in_maps.append(
            {
                "h16": np.ascontiguousarray(h16_full[sl]),
                "w1t": w1t,
                "w2t": w2t,
                "hdect": hdect,
                "b1t": b1t,
                "b12t": b12t,
                "vt": vt,
            }
        )

    res = run_bass_kernel_spmd(nc, in_maps, core_ids=list(range(NCORES)))
    outs = []
    for core in range(NCORES):
        o = res.results[core]["ctx_out"]  # [NB, P, NCH]
        outs.append(o.transpose(0, 2, 1).reshape(NB, D))  # e = c*128 + p
    return np.concatenate(outs, axis=0).astype(np.float32)
